# revision 31
# baseline (speedup 1.0000x reference)
"""Trainium2 Bass kernel for nn_ExtractionModel (NMS keypoint detection).

Self-contained: builds one SPMD Bass program, runs it on 8 NeuronCores via
run_bass_kernel_spmd, combines per-core partial outputs on the host.

Algorithm (per core):
  Phase A (replicated): NMS + Newton localization on the 3 score maps.
  Phase B (replicated): threshold-pruned candidate extraction (max8/match_replace)
    -> int32 keys (value<<12 | slot tie-break) -> bitonic top-2048 sort.
  Phase C (band-sharded): each core gathers/normalizes descriptors only for the
    winners inside its 72-row band of the 576-row virtual stack, scattering rows
    into the output by rank. Host sums the per-core partial kp/desc outputs.
"""
import numpy as np

H = W = 192
T0 = 0.974
M0 = 8170505  # ceil(T0 * 2^23)
FRAME = 74 * 192  # 14208
NCORES = 8

_CACHE = {}


def _build_program():
    from contextlib import ExitStack
    from concourse import bass, bacc, mybir, tile

    f32 = mybir.dt.float32
    i32 = mybir.dt.int32
    i16 = mybir.dt.int16
    u16 = mybir.dt.uint16
    u32 = mybir.dt.uint32
    A = mybir.AluOpType

    nc = bacc.Bacc(None, target_bir_lowering=False, debug=True)

    # ---------------- I/O ----------------
    scores_in = [nc.declare_dram_parameter(f"score{s}", [H, W], f32, isOutput=False)
                 for s in range(3)]
    feats_in = {(f, t): nc.declare_dram_parameter(f"feat{f}{t}", [128, FRAME], f32,
                                                  isOutput=False)
                for f in range(2) for t in range(2)}
    # per-core constants [128, 8]: c0=cboff0 c1=cboff1 c2=b0 c3=b1
    consts_in = nc.declare_dram_parameter("consts", [128, 8], f32, isOutput=False)

    out_kp = nc.declare_dram_parameter("out_kp", [2048, 2], f32, isOutput=True)
    out_desc = nc.declare_dram_parameter("out_desc", [2048, 256], f32, isOutput=True)
    out_scores = nc.declare_dram_parameter("out_scores", [2048], f32, isOutput=True)
    out_det = nc.declare_dram_parameter("out_det", [3, H, W], f32, isOutput=True)
    out_dbg = nc.declare_dram_parameter("out_dbg", [128, 32], mybir.dt.int32, isOutput=True)

    # ---------------- inline constants ----------------
    prow = nc.inline_tensor(np.arange(128, dtype=np.float32).reshape(128, 1), "prow")
    coliota = nc.inline_tensor(
        np.broadcast_to(np.arange(W, dtype=np.float32), (128, W)).copy(), "coliota")
    ident = nc.inline_tensor(np.eye(128, dtype=np.float32), "ident")
    ones128 = nc.inline_tensor(np.ones((128, 1), np.float32), "ones128")
    onesrow = nc.inline_tensor(np.ones((1, 128), np.float32), "onesrow")
    slotiota_np = (np.arange(128)[:, None] * 32 + np.arange(32)[None, :])
    slotiota = nc.inline_tensor(slotiota_np.astype(np.float32), "slotiota")
    e_np = np.arange(128)[:, None] * 32 + np.arange(32)[None, :]
    sgn_list = []
    for ki_ in range(1, 12):  # stages k=2..2048 (k=4096 is all +1)
        k = 2 ** ki_
        sgn_list.append(np.where((e_np & k) != 0, -1, 1).astype(np.float32))
    sgnmasks = nc.inline_tensor(np.concatenate(sgn_list, axis=1), "sgnmasks")  # [128, 352]
    rankiota = nc.inline_tensor(
        (np.arange(128)[:, None] * 32 + np.arange(32)[None, :]).astype(np.float32),
        "rankiota")
    rankrow = nc.inline_tensor(np.arange(2048, dtype=np.float32).reshape(1, 2048),
                               "rankrow")
    halfiota = nc.inline_tensor(np.arange(16, dtype=np.float32).reshape(16, 1),
                                "halfiota")
    onesr16 = nc.inline_tensor(np.ones((1, 16), np.float32), "onesr16")
    wiota16 = nc.inline_tensor(
        (np.arange(32)[None, :] * 16 + np.arange(16)[:, None]).astype(np.float32),
        "wiota16")

    with tile.TileContext(nc) as tc, ExitStack() as ctx:
        pconst = ctx.enter_context(tc.tile_pool(name="pconst", bufs=1))
        psum = ctx.enter_context(tc.tile_pool(name="psum", bufs=1, space="PSUM"))
        dram = ctx.enter_context(tc.tile_pool(name="dram", bufs=1, space="DRAM"))
        es1 = ExitStack()
        sb = es1.enter_context(tc.tile_pool(name="sbA", bufs=1))
        sb1 = es1.enter_context(tc.tile_pool(name="sb1A", bufs=1))
        sbd = es1.enter_context(tc.tile_pool(name="sbdA", bufs=2))

        V = nc.vector
        G = nc.gpsimd
        S = nc.scalar

        canddram = dram.tile([576, W], f32)
        kjud = dram.tile([576, W], f32)
        kiud = dram.tile([576, W], f32)
        cbd = dram.tile([576, W], f32)
        wid = dram.tile([576, W], f32)
        wjd = dram.tile([576, W], f32)
        locdram = dram.tile([128, 32], f32)
        mdram = dram.tile([4, 2048], f32)
        wdram = dram.tile([3, 512], f32)
        rkdram = dram.tile([512], f32)
        slotdram = dram.tile([2048], i32)
        seldram = dram.tile([2048], f32)
        lrowdram = dram.tile([2048], f32)
        lidxdram = dram.tile([2048], f32)
        cbcdram = dram.tile([16, 32], f32)
        descscr = dram.tile([2048, 256], f32)

        # load constants to SBUF
        constt = pconst.tile([128, 8], f32)
        nc.sync.dma_start(out=constt[:], in_=consts_in[:])
        prowt = pconst.tile([128, 1], f32)
        nc.sync.dma_start(out=prowt[:], in_=prow[:])
        colt = pconst.tile([128, W], f32)
        nc.sync.dma_start(out=colt[:], in_=coliota[:])
        identt = pconst.tile([128, 128], f32)
        nc.sync.dma_start(out=identt[:], in_=ident[:])
        ones_t = pconst.tile([128, 1], f32)
        nc.sync.dma_start(out=ones_t[:], in_=ones128[:])
        onesr_t = pconst.tile([1, 128], f32)
        nc.sync.dma_start(out=onesr_t[:], in_=onesrow[:])
        slotnt = sb1.tile([128, 32], f32)
        nc.sync.dma_start(out=slotnt[:], in_=slotiota[:])
        sgnt = sb1.tile([128, 352], f32)
        nc.sync.dma_start(out=sgnt[:], in_=sgnmasks[:])
        halfit = pconst.tile([16, 1], f32)
        nc.sync.dma_start(out=halfit[:], in_=halfiota[:])
        onesr16t = pconst.tile([1, 16], f32)
        nc.sync.dma_start(out=onesr16t[:], in_=onesr16[:])


        # ================= PHASE A =================
        for s in range(3):
            for v in range(2):
                r0 = 96 * v  # first interior image row of this vtile
                # three row-shifted, zero-padded copies (base partition 0)
                tls = {}
                for nm, shift in (("scC", 0), ("scU", 1), ("scD", -1)):
                    tl = sb.tile([96, W + 2], f32, tag=nm, name=nm)
                    V.memset(tl[:], 0.0)
                    lo = max(0, r0 + shift)
                    hi = min(192, r0 + 96 + shift)
                    plo = lo - (r0 + shift)
                    nc.sync.dma_start(out=tl[plo:plo + (hi - lo), 1:W + 1],
                                      in_=scores_in[s][lo:hi, :])
                    tls[nm] = tl
                scC, scU, scD = tls["scC"], tls["scU"], tls["scD"]
                inter = scC[:, 1:W + 1]

                h3c = sb.tile([96, W], f32, tag="h3c")
                V.tensor_max(h3c[:], scC[:, 0:W], scC[:, 1:W + 1])
                V.tensor_max(h3c[:], h3c[:], scC[:, 2:W + 2])
                h3u = sb.tile([96, W], f32, tag="h3u")
                V.tensor_max(h3u[:], scU[:, 0:W], scU[:, 1:W + 1])
                V.tensor_max(h3u[:], h3u[:], scU[:, 2:W + 2])
                lm = sb.tile([96, W], f32, tag="lm")
                V.tensor_max(lm[:], scD[:, 0:W], scD[:, 1:W + 1])
                V.tensor_max(lm[:], lm[:], scD[:, 2:W + 2])
                V.tensor_max(lm[:], lm[:], h3u[:])
                V.tensor_max(lm[:], lm[:], h3c[:])
                eq = sb.tile([96, W], f32, tag="eq")
                V.tensor_tensor(eq[:], lm[:], inter, op=A.is_equal)
                det = sb.tile([96, W], f32, tag="det")
                V.tensor_mul(det[:], inter, eq[:])
                nc.sync.dma_start(out=out_det[s, r0:r0 + 96, :], in_=det[:])

                di = sb.tile([96, W], f32, tag="di")
                V.tensor_sub(di[:], scU[:, 1:W + 1], scD[:, 1:W + 1])
                V.tensor_scalar_mul(di[:], di[:], 0.5)
                dj = sb.tile([96, W], f32, tag="dj")
                V.tensor_sub(dj[:], scC[:, 2:W + 2], scC[:, 0:W])
                V.tensor_scalar_mul(dj[:], dj[:], 0.5)
                dii = sb.tile([96, W], f32, tag="dii")
                V.tensor_add(dii[:], scD[:, 1:W + 1], scU[:, 1:W + 1])
                V.scalar_tensor_tensor(out=dii[:], in0=inter, scalar=-2.0,
                                       in1=dii[:], op0=A.mult, op1=A.add)
                djj = sb.tile([96, W], f32, tag="djj")
                V.tensor_add(djj[:], scC[:, 0:W], scC[:, 2:W + 2])
                V.scalar_tensor_tensor(out=djj[:], in0=inter, scalar=-2.0,
                                       in1=djj[:], op0=A.mult, op1=A.add)
                dij = sb.tile([96, W], f32, tag="dij")
                tb = sb.tile([96, W], f32, tag="tb")
                V.tensor_sub(dij[:], scD[:, 0:W], scD[:, 2:W + 2])
                V.tensor_sub(tb[:], scU[:, 2:W + 2], scU[:, 0:W])
                V.tensor_add(dij[:], dij[:], tb[:])
                V.tensor_scalar_mul(dij[:], dij[:], 0.25)

                dd = sb.tile([96, W], f32, tag="dd")
                V.tensor_mul(dd[:], dii[:], djj[:])
                V.tensor_mul(tb[:], dij[:], dij[:])
                V.tensor_sub(dd[:], dd[:], tb[:])
                ddz = sb.tile([96, W], f32, tag="ddz")
                V.tensor_scalar(ddz[:], dd[:], 0.0, None, op0=A.is_equal)
                V.scalar_tensor_tensor(out=dd[:], in0=ddz[:], scalar=1e-20,
                                       in1=dd[:], op0=A.mult, op1=A.add)
                rcp = sb.tile([96, W], f32, tag="rcp")
                V.reciprocal(rcp[:], dd[:])

                si = sb.tile([96, W], f32, tag="si")
                V.tensor_mul(si[:], djj[:], di[:])
                V.tensor_mul(tb[:], dij[:], dj[:])
                V.tensor_sub(si[:], tb[:], si[:])
                V.tensor_mul(si[:], si[:], rcp[:])
                sj = sb.tile([96, W], f32, tag="sj")
                V.tensor_mul(sj[:], dii[:], dj[:])
                V.tensor_mul(tb[:], dij[:], di[:])
                V.tensor_sub(sj[:], tb[:], sj[:])
                V.tensor_mul(sj[:], sj[:], rcp[:])

                valid = sb.tile([96, W], f32, tag="valid")
                V.tensor_scalar(valid[:], det[:], 0.0, None, op0=A.not_equal)
                S.activation(out=tb[:], in_=si[:],
                             func=mybir.ActivationFunctionType.Abs)
                V.tensor_scalar(tb[:], tb[:], 0.5, None, op0=A.is_lt)
                V.tensor_mul(valid[:], valid[:], tb[:])
                S.activation(out=tb[:], in_=sj[:],
                             func=mybir.ActivationFunctionType.Abs)
                V.tensor_scalar(tb[:], tb[:], 0.5, None, op0=A.is_lt)
                V.tensor_mul(valid[:], valid[:], tb[:])

                iiS = sb.tile([96, 1], f32, tag="iiS")
                V.tensor_scalar(iiS[:], prowt[0:96, :], float(r0), None, op0=A.add)
                ki = sb.tile([96, W], f32, tag="ki")
                V.tensor_scalar(ki[:], si[:], iiS[:], None, op0=A.add)
                V.tensor_mul(ki[:], ki[:], valid[:])
                kj = sb.tile([96, W], f32, tag="kj")
                V.tensor_add(kj[:], sj[:], colt[0:96, :])
                V.tensor_mul(kj[:], kj[:], valid[:])

                i0 = sb.tile([96, W], f32, tag="i0")
                V.tensor_scalar(i0[:], ki[:], iiS[:], None, op0=A.is_lt)
                V.tensor_scalar(i0[:], i0[:], -1.0, iiS[:], op0=A.mult, op1=A.add)
                j0 = sb.tile([96, W], f32, tag="j0")
                V.tensor_tensor(j0[:], kj[:], colt[0:96, :], op=A.is_lt)
                V.scalar_tensor_tensor(out=j0[:], in0=j0[:], scalar=-1.0,
                                       in1=colt[0:96, :], op0=A.mult, op1=A.add)
                V.tensor_scalar(tb[:], i0[:], 0.0, None, op0=A.is_ge)
                V.tensor_mul(valid[:], valid[:], tb[:])
                V.tensor_scalar(tb[:], i0[:], float(H - 2), None, op0=A.is_le)
                V.tensor_mul(valid[:], valid[:], tb[:])
                V.tensor_scalar(tb[:], j0[:], 0.0, None, op0=A.is_ge)
                V.tensor_mul(valid[:], valid[:], tb[:])
                V.tensor_scalar(tb[:], j0[:], float(W - 2), None, op0=A.is_le)
                V.tensor_mul(valid[:], valid[:], tb[:])

                V.tensor_scalar(i0[:], i0[:], 0.0, float(H - 2), op0=A.max, op1=A.min)
                V.tensor_scalar(j0[:], j0[:], 0.0, float(W - 2), op0=A.max, op1=A.min)
                wi = sb.tile([96, W], f32, tag="wi")
                V.tensor_sub(wi[:], ki[:], i0[:])
                wj = sb.tile([96, W], f32, tag="wj")
                V.tensor_sub(wj[:], kj[:], j0[:])
                cb = sb.tile([96, W], f32, tag="cb")
                V.tensor_scalar(cb[:], i0[:], 192.0, float(s * 36864),
                                op0=A.mult, op1=A.add)
                V.tensor_add(cb[:], cb[:], j0[:])
                kju = sb.tile([96, W], f32, tag="kju")
                V.tensor_scalar(kju[:], kj[:], 16.0, 7.5, op0=A.mult, op1=A.add)
                kiu = sb.tile([96, W], f32, tag="kiu")
                V.tensor_scalar(kiu[:], ki[:], 16.0, 7.5, op0=A.mult, op1=A.add)

                c1 = sb.tile([96, W], f32, tag="c1")
                V.tensor_scalar(c1[:], det[:], T0, None, op0=A.is_gt)
                V.tensor_mul(c1[:], c1[:], valid[:])
                cand = sb.tile([96, W], f32, tag="cand")
                V.tensor_mul(cand[:], det[:], c1[:])
                V.tensor_add(cand[:], cand[:], c1[:])
                V.tensor_scalar(cand[:], cand[:], 1.0, None, op0=A.subtract)

                row0 = s * 192 + r0
                nc.sync.dma_start(out=canddram[row0:row0 + 96, :], in_=cand[:])
                nc.sync.dma_start(out=kjud[row0:row0 + 96, :], in_=kju[:])
                nc.sync.dma_start(out=kiud[row0:row0 + 96, :], in_=kiu[:])
                nc.sync.dma_start(out=cbd[row0:row0 + 96, :], in_=cb[:])
                nc.sync.dma_start(out=wid[row0:row0 + 96, :], in_=wi[:])
                nc.sync.dma_start(out=wjd[row0:row0 + 96, :], in_=wj[:])

        # ================= PHASE B: extraction =================
        candt = sb1.tile([128, 864], f32)
        nc.sync.dma_start(out=candt[:], in_=canddram[:].rearrange("a b -> (a b)")
                          .rearrange("(p f) -> p f", p=128))
        vals = sb1.tile([128, 32], f32)
        locs = sb1.tile([128, 32], f32)
        p864 = sb1.tile([128, 1], f32)
        V.tensor_scalar(p864[:], prowt[:], 864.0, None, op0=A.mult)
        for h in range(2):
            work = sb.tile([128, 432], f32, tag="work")
            V.tensor_copy(work[:], candt[:, 432 * h:432 * h + 432])
            for r in range(2):
                mx = sb.tile([128, 8], f32, tag="mx")
                V.max(out=mx[:], in_=work[:])
                mi = sb.tile([128, 8], u16, tag="mi")
                V.max_index(mi[:], mx[:], work[:])
                V.tensor_copy(vals[:, 16 * h + 8 * r:16 * h + 8 * r + 8], mx[:])
                mif = sb.tile([128, 8], f32, tag="mif")
                V.tensor_copy(mif[:], mi[:])
                V.tensor_scalar(locs[:, 16 * h + 8 * r:16 * h + 8 * r + 8],
                                mif[:], p864[:], float(432 * h),
                                op0=A.add, op1=A.add)
                if r == 0:
                    work2 = sb.tile([128, 432], f32, tag="work2")
                    V.match_replace(out=work2[:], in_to_replace=mx[:],
                                    in_values=work[:], imm_value=-1.0)
                    work = work2
        nc.sync.dma_start(out=locdram[:], in_=locs[:])

        # keys
        key = sbd.tile([128, 32], f32, tag="key128")
        V.tensor_scalar(key[:], vals[:], float(2.0 ** 23), float(-M0),
                        op0=A.mult, op1=A.add)
        V.tensor_scalar_max(key[:], key[:], -1.0)
        pay = sbd.tile([128, 32], f32, tag="pay128")
        V.tensor_copy(pay[:], slotnt[:])

        # ================= bitonic sort (descending, key+payload) =================
        def pass_free(sk, sp, j, P, F):
            # compare-exchange at free distance j on [P, F] tiles
            dk = sbd.tile([P, F], f32, tag=f"key{P}", name="dk")
            dp = sbd.tile([P, F], f32, tag=f"pay{P}", name="dp")
            sv = sk[:].rearrange("p (g two r) -> p g two r", two=2, r=j)
            pv = sp[:].rearrange("p (g two r) -> p g two r", two=2, r=j)
            dv = dk[:].rearrange("p (g two r) -> p g two r", two=2, r=j)
            qv = dp[:].rearrange("p (g two r) -> p g two r", two=2, r=j)
            pr = sb.tile([P, F], i32, tag=f"pr{P}", name="pr")
            prv = pr[:].rearrange("p (g two r) -> p g two r", two=2, r=j)[:, :, 0, :]
            V.tensor_tensor(prv[:], sv[:, :, 0, :], sv[:, :, 1, :], op=A.is_ge)
            V.tensor_tensor(dv[:, :, 0, :], sv[:, :, 0, :], sv[:, :, 1, :], op=A.max)
            V.tensor_tensor(dv[:, :, 1, :], sv[:, :, 0, :], sv[:, :, 1, :], op=A.min)
            V.tensor_copy(qv[:, :, 0, :], pv[:, :, 1, :])
            V.copy_predicated(qv[:, :, 0, :], prv[:], pv[:, :, 0, :])
            V.tensor_copy(qv[:, :, 1, :], pv[:, :, 0, :])
            V.copy_predicated(qv[:, :, 1, :], prv[:], pv[:, :, 1, :])
            return dk, dp

        def pe_transpose(src_ap, P, F, tagn):
            # [P, F] -> [F, P] via PE transpose
            pst = psum.tile([F, P], f32, tag=f"pst{F}", name="pst")
            nc.tensor.transpose(out=pst[:], in_=src_ap, identity=identt[0:P, 0:P])
            ot = sbd.tile([F, P], f32, tag=tagn, name="ot")
            V.tensor_copy(ot[:], pst[:])
            return ot

        cur, curp = key, pay
        for ki_ in range(1, 13):
            k = 2 ** ki_
            if k < 4096:
                sg = sgnt[:, 32 * (ki_ - 1):32 * ki_]
                nxt = sbd.tile([128, 32], f32, tag="key128", name="nxt")
                V.tensor_mul(nxt[:], cur[:], sg)
                cur = nxt
            js = []
            j = k // 2
            while j >= 1:
                js.append(j)
                j //= 2
            jbig = [j for j in js if j >= 1024]
            jmid = [j for j in js if 32 <= j <= 512]
            jsml = [j for j in js if j <= 16]
            if jbig:
                Tk = pe_transpose(cur[:], 128, 32, "keyT")
                Tp = pe_transpose(curp[:], 128, 32, "payT")
                for j in jbig:
                    Tk, Tp = pass_free(Tk, Tp, j // 32, 32, 128)
                cur = pe_transpose(Tk[:], 32, 128, "key128")
                curp = pe_transpose(Tp[:], 32, 128, "pay128")
            if jmid:
                Bk = sbd.tile([128, 32], f32, tag="key128", name="Bk")
                V.transpose(Bk[:], cur[:])
                Bp = sbd.tile([128, 32], f32, tag="pay128", name="Bp")
                V.transpose(Bp[:], curp[:])
                cur, curp = Bk, Bp
                for j in jmid:
                    cur, curp = pass_free(cur, curp, j // 32, 128, 32)
                Bk2 = sbd.tile([128, 32], f32, tag="key128", name="Bk2")
                V.transpose(Bk2[:], cur[:])
                Bp2 = sbd.tile([128, 32], f32, tag="pay128", name="Bp2")
                V.transpose(Bp2[:], curp[:])
                cur, curp = Bk2, Bp2
            for j in jsml:
                cur, curp = pass_free(cur, curp, j, 128, 32)
            if k < 4096:
                sg = sgnt[:, 32 * (ki_ - 1):32 * ki_]
                nxt = sbd.tile([128, 32], f32, tag="key128", name="nxt2")
                V.tensor_mul(nxt[:], cur[:], sg)
                cur = nxt

        # ============ tie fix: ascending slots within equal-key runs ============
        kL = sb1.tile([128, 32], f32)
        V.memset(kL[:, 0:1], 3.0e8)
        V.tensor_copy(kL[:, 1:32], cur[:, 0:31])
        nc.sync.dma_start(out=kL[1:128, 0:1], in_=cur[0:127, 31:32])
        pL = sb1.tile([128, 32], f32)
        V.memset(pL[:, 0:1], 0.0)
        V.tensor_copy(pL[:, 1:32], curp[:, 0:31])
        nc.sync.dma_start(out=pL[1:128, 0:1], in_=curp[0:127, 31:32])
        kR = sb1.tile([128, 32], f32)
        V.memset(kR[:, 31:32], -2.0)
        V.tensor_copy(kR[:, 0:31], cur[:, 1:32])
        nc.sync.dma_start(out=kR[0:127, 31:32], in_=cur[1:128, 0:1])
        pR = sb1.tile([128, 32], f32)
        V.memset(pR[:, 31:32], 0.0)
        V.tensor_copy(pR[:, 0:31], curp[:, 1:32])
        nc.sync.dma_start(out=pR[0:127, 31:32], in_=curp[1:128, 0:1])
        eqL = sb1.tile([128, 32], i32)
        V.tensor_tensor(eqL[:], cur[:], kL[:], op=A.is_equal)
        eqR = sb1.tile([128, 32], i32)
        V.tensor_tensor(eqR[:], cur[:], kR[:], op=A.is_equal)
        tmpn = sb1.tile([128, 32], f32)
        V.tensor_tensor(tmpn[:], curp[:], pR[:], op=A.min)
        tmpx = sb1.tile([128, 32], f32)
        V.tensor_tensor(tmpx[:], curp[:], pL[:], op=A.max)
        newp = sb1.tile([128, 32], f32)
        V.tensor_copy(newp[:], curp[:])
        V.copy_predicated(newp[:], eqR[:], tmpn[:])
        V.copy_predicated(newp[:], eqL[:], tmpx[:])
        curp = newp

        # ================= decode winners =================
        wk = cur[0:64, :]
        vf = sb1.tile([64, 32], f32)
        V.tensor_scalar(vf[:], wk, float(M0), None, op0=A.add)
        V.tensor_scalar(vf[:], vf[:], float(2.0 ** -23), None, op0=A.mult)
        sel = sb1.tile([64, 32], f32)
        V.tensor_scalar(sel[:], wk, 0.0, None, op0=A.is_ge)
        scw = sb1.tile([64, 32], f32)
        V.tensor_mul(scw[:], vf[:], sel[:])
        nc.sync.dma_start(out=out_scores[:], in_=scw[:])

        slotf = sb1.tile([64, 32], f32)
        V.tensor_mul(slotf[:], curp[0:64, :], sel[:])
        slott = sb1.tile([64, 32], i32)
        V.tensor_copy(slott[:], slotf[:])
        nc.sync.dma_start(out=out_dbg[0:64, :], in_=slott[:])

        nc.sync.dma_start(out=slotdram[:], in_=slott[:])
        nc.sync.dma_start(out=seldram[:], in_=sel[:])

        # ======== PHASE 2: winner attribute gathers (row space) ========
        es1.close()
        es2 = ExitStack()
        sb = es2.enter_context(tc.tile_pool(name="sbB", bufs=1))
        sb1 = es2.enter_context(tc.tile_pool(name="sb1B", bufs=1))
        rankrt = sb1.tile([1, 2048], f32)
        nc.sync.dma_start(out=rankrt[:], in_=rankrow[:])
        sw32 = sb1.tile([16, 128], i32)
        nc.sync.dma_start(out=sw32[:],
                          in_=slotdram[:].rearrange("(f p) -> p f", p=16))
        sw16 = sb1.tile([16, 128], i16)
        V.tensor_copy(sw16[:], sw32[:])

        # replicate loc list to 16 partitions
        loc1 = sb1.tile([16, 4096], f32)
        nc.sync.dma_start(out=loc1[0:1, :],
                          in_=locdram[:].rearrange("a b -> (a b)"))
        for db in (1, 2, 4, 8):
            nc.sync.dma_start(out=loc1[db:2 * db, :], in_=loc1[0:db, :])

        lw16 = sb1.tile([16, 2048], f32)
        G.ap_gather(out_ap=lw16[:].unsqueeze(2), in_ap=loc1[:].unsqueeze(2),
                    idxs_ap=sw16[:], channels=16, num_elems=4096, d=1,
                    num_idxs=2048)
        lrow = lw16[0:1, :]

        # half index h in [0,6) and local index within half
        hrow = sb1.tile([1, 2048], f32)
        V.tensor_scalar(hrow[:], lrow, 9216.0, None, op0=A.is_ge)
        for kk in range(2, 12):
            htmp = sb.tile([1, 2048], f32, tag="rtmp", name="htmp", bufs=2)
            V.tensor_scalar(htmp[:], lrow, float(9216 * kk), None, op0=A.is_ge)
            V.tensor_add(hrow[:], hrow[:], htmp[:])
        lidxrow = sb1.tile([1, 2048], f32)
        V.tensor_scalar(lidxrow[:], hrow[:], -9216.0, None, op0=A.mult)
        V.tensor_add(lidxrow[:], lidxrow[:], lrow)
        nc.sync.dma_start(out=lidxdram[:], in_=lidxrow[:])
        liw32 = sb1.tile([16, 128], f32)
        nc.sync.dma_start(out=liw32[:],
                          in_=lidxdram[:].rearrange("(f p) -> p f", p=16))
        liw16 = sb1.tile([16, 128], i16)
        V.tensor_copy(liw16[:], liw32[:])

        # sel to row space
        selrow = sb1.tile([1, 2048], f32)
        nc.sync.dma_start(out=selrow[:], in_=seldram[:])

        ownrow = sb.tile([1, 2048], f32, tag="rtmp", name="ownrow", bufs=2)
        owntmp = sb.tile([1, 2048], f32, tag="rtmp", name="owntmp", bufs=2)
        V.tensor_scalar(ownrow[:], lrow, constt[0:1, 2:3], None, op0=A.is_ge)
        V.tensor_scalar(owntmp[:], lrow, constt[0:1, 3:4], None, op0=A.is_lt)
        V.tensor_mul(ownrow[:], ownrow[:], owntmp[:])
        omrow = sb1.tile([1, 2048], f32)
        V.tensor_mul(omrow[:], ownrow[:], selrow[:])

        # broadcast h to 16 partitions via PE; build half mask
        hb = sb1.tile([16, 2048], f32)
        for ch in range(4):
            psb = psum.tile([16, 512], f32, tag="psb", name="psb")
            nc.tensor.matmul(out=psb[:], lhsT=onesr16t[:],
                             rhs=hrow[:, 512 * ch:512 * ch + 512],
                             start=True, stop=True)
            V.tensor_copy(hb[:, 512 * ch:512 * ch + 512], psb[:])
        hmask = sb1.tile([16, 2048], f32)
        V.tensor_scalar(hmask[:], hb[:], halfit[:], None, op0=A.is_equal)

        def attr_gather_row(src_dram, name):
            at = sb.tile([16, 9216], f32, tag="attrin", name="attrin")
            nc.sync.dma_start(
                out=at[0:12, :],
                in_=src_dram[:].rearrange("(h r) c -> h (r c)", h=12))
            nc.sync.dma_start(
                out=at[12:16, :],
                in_=src_dram[:].rearrange("(h r) c -> h (r c)", h=12)[0:4, :])
            g16 = sb.tile([16, 2048], f32, tag="attrg", name="attrg")
            G.ap_gather(out_ap=g16[:].unsqueeze(2), in_ap=at[:].unsqueeze(2),
                        idxs_ap=liw16[:], channels=16, num_elems=9216, d=1,
                        num_idxs=2048)
            V.tensor_mul(g16[:], g16[:], hmask[:])
            row = sb.tile([1, 2048], f32, tag="arow", name=f"arow_{name}", bufs=2)
            for ch in range(4):
                psr = psum.tile([1, 512], f32, tag="psr", name="psr")
                nc.tensor.matmul(out=psr[:], lhsT=ones_t[0:16, :],
                                 rhs=g16[:, 512 * ch:512 * ch + 512],
                                 start=True, stop=True)
                V.tensor_copy(row[:, 512 * ch:512 * ch + 512], psr[:])
            return row

        def mask_write(src_row, k):
            t = sb.tile([1, 2048], f32, tag="rtmp", name="mw", bufs=2)
            V.tensor_mul(t[:], src_row, omrow[:])
            V.tensor_add(t[:], t[:], omrow[:])
            V.tensor_scalar(t[:], t[:], 1.0, None, op0=A.subtract)
            nc.sync.dma_start(out=mdram[k, :], in_=t[:])

        row = attr_gather_row(kjud, "kju")
        kpr = sb.tile([1, 2048], f32, tag="rtmp", name="kpr", bufs=2)
        V.tensor_mul(kpr[:], row[:], omrow[:])
        nc.sync.dma_start(out=out_kp[:, 0:1], in_=kpr[:])
        row = attr_gather_row(kiud, "kiu")
        kpr = sb.tile([1, 2048], f32, tag="rtmp", name="kpr2", bufs=2)
        V.tensor_mul(kpr[:], row[:], omrow[:])
        nc.sync.dma_start(out=out_kp[:, 1:2], in_=kpr[:])
        row = attr_gather_row(cbd, "cb")
        mask_write(row[:], 0)
        mask_write(rankrt[:], 1)
        row = attr_gather_row(wid, "wi")
        mask_write(row[:], 2)
        row = attr_gather_row(wjd, "wj")
        mask_write(row[:], 3)

        # read back wrapped [16,128], sparse-compact to [16,32]
        wiotat = sb1.tile([16, 32], f32)
        nc.sync.dma_start(out=wiotat[:], in_=wiota16[:])
        comp = {}
        for k, name in ((0, "cb"), (1, "rk"), (2, "wi"), (3, "wj")):
            t = sb.tile([16, 128], f32, tag="cmpin")
            nc.sync.dma_start(out=t[:],
                              in_=mdram[k, :].rearrange("(f p) -> p f", p=16))
            o = sb1.tile([16, 32], f32, tag=f"comp_{name}")
            nf = sb.tile([1, 1], u32, tag="nf")
            G.sparse_gather(out=o[:], in_=t[:], num_found=nf[:])
            # deterministic -1 padding: positions >= num_found forced to -1
            nff = sb.tile([1, 1], f32, tag="nff")
            V.tensor_copy(nff[:], nf[:])
            nf16 = sb.tile([16, 1], f32, tag="nf16")
            nc.sync.dma_start(out=nf16[0:1, :], in_=nff[:])
            for db in (1, 2, 4, 8):
                nc.sync.dma_start(out=nf16[db:2 * db, :], in_=nf16[0:db, :])
            pdm = sb.tile([16, 32], i32, tag="pdm")
            V.tensor_scalar(pdm[:], wiotat[:], nf16[:], None, op0=A.is_lt)
            o2 = sb1.tile([16, 32], f32, tag=f"comp2_{name}", name=f"o2_{name}")
            V.memset(o2[:], -1.0)
            V.copy_predicated(o2[:], pdm[:], o[:])
            comp[name] = o2

        # rank offsets for the scatter: pad -> 60000 so bounds check skips
        rz = sb1.tile([16, 32], f32)
        V.tensor_scalar(rz[:], comp["rk"][:], 0.0, None, op0=A.is_lt)
        V.scalar_tensor_tensor(out=rz[:], in0=rz[:], scalar=60001.0,
                               in1=comp["rk"][:], op0=A.mult, op1=A.add)
        nc.sync.dma_start(out=rkdram[:].rearrange("(f p) -> p f", p=16), in_=rz[:])
        # weights roundtrip -> wdram, cbc handoff
        for k, name in ((0, "wi"), (1, "wj"), (2, "cb")):
            nc.sync.dma_start(out=wdram[k, :].rearrange("(f p) -> p f", p=16),
                              in_=comp[name][:])
        nc.sync.dma_start(out=cbcdram[:], in_=comp["cb"][:])

        # ======== PHASE 3: descriptors ========
        es2.close()
        es3 = ExitStack()
        sb = es3.enter_context(tc.tile_pool(name="sbC", bufs=1))
        sb1 = es3.enter_context(tc.tile_pool(name="sb1C", bufs=1))
        featp = es3.enter_context(tc.tile_pool(name="featp", bufs=1))
        cbct = sb1.tile([16, 32], f32)
        nc.sync.dma_start(out=cbct[:], in_=cbcdram[:])

        # corner index tiles per frag: [16,128] int16 wrapped, replicated to [128,128]
        idx16 = {}
        for f in range(2):
            base = sb.tile([16, 32], f32, tag="cbase")
            V.tensor_scalar(base[:], cbct[:], constt[0:16, f:f + 1], None,
                            op0=A.subtract)
            V.tensor_scalar(base[:], base[:], 0.0, float(FRAME - 194),
                            op0=A.max, op1=A.min)
            it = sb1.tile([128, 128], i16, tag=f"idx{f}")
            for ci, off in enumerate((0, 1, 192, 193)):
                cf = sb.tile([16, 32], f32, tag="cf")
                V.tensor_scalar(cf[:], base[:], float(off), None, op0=A.add)
                V.tensor_copy(it[0:16, 32 * ci:32 * ci + 32], cf[:])
            for rep in range(1, 8):
                nc.sync.dma_start(out=it[16 * rep:16 * rep + 16, :],
                                  in_=it[0:16, :])
            idx16[f] = it

        # weights rows from wdram
        wrow = [sb1.tile([1, 512], f32, tag=f"wrow{k}", name=f"wrow{k}")
                for k in range(3)]
        for k in range(3):
            nc.sync.dma_start(out=wrow[k][:], in_=wdram[k:k + 1, :])
        aw = sb1.tile([1, 512], f32)
        V.tensor_scalar(aw[:], wrow[0][:], -1.0, 1.0, op0=A.mult, op1=A.add)
        bw = sb1.tile([1, 512], f32)
        V.tensor_scalar(bw[:], wrow[1][:], -1.0, 1.0, op0=A.mult, op1=A.add)
        w4 = [sb1.tile([1, 512], f32, tag=f"w4_{ci}", name=f"w4_{ci}") for ci in range(4)]
        V.tensor_mul(w4[0][:], aw[:], bw[:])
        V.tensor_mul(w4[1][:], aw[:], wrow[1][:])
        V.tensor_mul(w4[2][:], wrow[0][:], bw[:])
        V.tensor_mul(w4[3][:], wrow[0][:], wrow[1][:])
        pmrow = sb1.tile([1, 512], f32)
        V.tensor_scalar(pmrow[:], wrow[2][:], 0.0, None, op0=A.is_ge)
        fm = [sb1.tile([1, 512], f32, tag=f"fm_{f}", name=f"fm_{f}") for f in range(2)]
        # fm_f = (0 <= cb - off) & (cb - off <= FRAME-194) & pm
        for f in range(2):
            t1 = sb.tile([1, 512], f32, tag="fmt")
            V.tensor_scalar(t1[:], wrow[2][:], constt[0:1, f:f + 1], None,
                            op0=A.subtract)
            t2 = sb.tile([1, 512], f32, tag="fmt2")
            V.tensor_scalar(t2[:], t1[:], 0.0, None, op0=A.is_ge)
            V.tensor_scalar(t1[:], t1[:], float(FRAME - 194), None, op0=A.is_le)
            V.tensor_mul(t1[:], t1[:], t2[:])
            V.tensor_mul(fm[f][:], t1[:], pmrow[:])

        # ================= descriptor gather + combine =================
        def bcast_row(row_ap, tagn):
            ps_b = psum.tile([128, 512], f32, tag="bps")
            nc.tensor.matmul(out=ps_b[:], lhsT=onesr_t[:], rhs=row_ap,
                             start=True, stop=True)
            ob = sb1.tile([128, 512], f32, tag=tagn)
            V.tensor_copy(ob[:], ps_b[:])
            return ob

        w4b = [bcast_row(w4[ci][:], f"w4b{ci}") for ci in range(4)]
        fmb = [bcast_row(fm[f][:], f"fmb{f}") for f in range(2)]
        desct = []
        for t in range(2):
            dt_ = sb1.tile([128, 512], f32, tag=f"desc{t}")
            V.memset(dt_[:], 0.0)
            desct.append(dt_)
        for f in range(2):
            ftiles = {}
            for t in range(2):
                ft = featp.tile([128, FRAME], f32, tag="featc")
                nc.sync.dma_start(out=ft[:], in_=feats_in[(f, t)][:])
                ftiles[t] = ft
            for t in range(2):
                g = sb.tile([128, 2048], f32, tag="gat")
                G.ap_gather(out_ap=g[:].unsqueeze(2), in_ap=ftiles[t][:].unsqueeze(2),
                            idxs_ap=idx16[f][:], channels=128, num_elems=FRAME,
                            d=1, num_idxs=2048)
                comb = sb.tile([128, 512], f32, tag="comb")
                V.tensor_mul(comb[:], g[:, 0:512], w4b[0][:])
                for ci in range(1, 4):
                    t3 = sb.tile([128, 512], f32, tag="combt")
                    V.tensor_mul(t3[:], g[:, 512 * ci:512 * ci + 512], w4b[ci][:])
                    V.tensor_add(comb[:], comb[:], t3[:])
                V.tensor_mul(comb[:], comb[:], fmb[f][:])
                V.tensor_add(desct[t][:], desct[t][:], comb[:])

        # ================= normalize =================
        nps = psum.tile([1, 512], f32, tag="nps")
        for t in range(2):
            sq = sb.tile([128, 512], f32, tag="sq")
            S.activation(out=sq[:], in_=desct[t][:],
                         func=mybir.ActivationFunctionType.Square)
            nc.tensor.matmul(out=nps[:], lhsT=ones_t[:], rhs=sq[:],
                             start=(t == 0), stop=(t == 1))
        nrm = sb1.tile([1, 512], f32)
        V.tensor_copy(nrm[:], nps[:])
        S.activation(out=nrm[:], in_=nrm[:], func=mybir.ActivationFunctionType.Sqrt)
        V.tensor_scalar_max(nrm[:], nrm[:], 1e-12)
        rcpn = sb1.tile([1, 512], f32)
        V.reciprocal(rcpn[:], nrm[:])
        rcb = bcast_row(rcpn[:], "rcb")
        for t in range(2):
            V.tensor_mul(desct[t][:], desct[t][:], rcb[:])

        # ================= transpose + rank scatter =================
        rkt = sb1.tile([128, 4], f32)
        nc.sync.dma_start(out=rkt[:], in_=rkdram[:].rearrange("(f p) -> p f", p=128))
        rki = sb1.tile([128, 4], i32)
        V.tensor_copy(rki[:], rkt[:])

        # zero-fill descriptor scratch
        dflat = descscr[:].rearrange("a b -> (a b)")
        for hh in range(2):
            zt = sb.tile([128, 2048], f32, tag="big")
            V.memset(zt[:], 0.0)
            nc.sync.dma_start(
                out=dflat[262144 * hh:262144 * (hh + 1)].rearrange(
                    "(p f) -> p f", p=128), in_=zt[:])

        for q in range(4):  # winner quarter (128 ranks each)
            ps = psum.tile([128, 128], f32, tag="pst")
            dT = sb.tile([128, 256], f32, tag="dT")
            for t in range(2):
                nc.tensor.transpose(out=ps[:], in_=desct[t][:, 128 * q:128 * q + 128],
                                    identity=identt[:])
                V.tensor_copy(dT[:, 128 * t:128 * t + 128], ps[:])
            G.indirect_dma_start(
                out=descscr[:], out_offset=bass.IndirectOffsetOnAxis(
                    ap=rki[:, q:q + 1], axis=0),
                in_=dT[:], in_offset=None,
                bounds_check=2047, oob_is_err=False)

        # copy scratch -> output
        oflat = out_desc[:].rearrange("a b -> (a b)")
        for hh in range(2):
            fin = sb.tile([128, 2048], f32, tag="big")
            nc.sync.dma_start(
                out=fin[:], in_=dflat[262144 * hh:262144 * (hh + 1)].rearrange(
                    "(p f) -> p f", p=128))
            nc.sync.dma_start(
                out=oflat[262144 * hh:262144 * (hh + 1)].rearrange(
                    "(p f) -> p f", p=128), in_=fin[:])
        es3.close()

    nc.compile()
    return nc


def _make_inputs_per_core(inputs):
    feats = [np.ascontiguousarray(inputs[k][0]) for k in
             ("feat_early", "feat_middle", "feat_deep")]
    smaps = [np.ascontiguousarray(inputs[k][0, 0], dtype=np.float32) for k in
             ("score_early", "score_middle", "score_deep")]
    in_maps = []
    for c in range(NCORES):
        m = {f"score{s}": smaps[s] for s in range(3)}
        r0 = 72 * c
        s0, off0 = r0 // 192, r0 % 192
        n1 = min(192 - off0, 72)
        frags = [(s0, off0)]
        frags.append((s0 + 1, 0) if n1 < 72 else (None, None))
        cboffs = []
        for (scl, off) in frags:
            if scl is None or scl > 2:
                cboffs.append(np.float32(1e9))
            else:
                cboffs.append(np.float32(scl * 36864 + (off - 1) * 192))
        for f, (scl, off) in enumerate(frags):
            if scl is None or scl > 2:
                fr = np.zeros((256, 74, 192), np.float32)
            else:
                fr = np.zeros((256, 74, 192), np.float32)
                lo, hi_r = off - 1, off + 73
                slo, shi = max(lo, 0), min(hi_r, 192)
                fr[:, slo - lo: slo - lo + (shi - slo)] = feats[scl][:, slo:shi]
            fr = fr.reshape(256, FRAME)
            for t in range(2):
                m[f"feat{f}{t}"] = np.ascontiguousarray(fr[128 * t:128 * t + 128])
        consts = np.zeros((128, 8), np.float32)
        consts[:, 0] = cboffs[0]
        consts[:, 1] = cboffs[1]
        consts[:, 2] = np.float32(13824 * c)
        consts[:, 3] = np.float32(13824 * (c + 1))
        m["consts"] = consts
        in_maps.append(m)
    return in_maps


def kernel(**inputs):
    if "nc" not in _CACHE:
        _CACHE["nc"] = _build_program()
    nc = _CACHE["nc"]
    in_maps = _make_inputs_per_core(inputs)
    from concourse.bass_utils import run_bass_kernel_spmd
    import os
    res = run_bass_kernel_spmd(nc, in_maps, list(range(NCORES)),
                               trace=bool(os.environ.get("KERNEL_TRACE")))
    _CACHE["last_result"] = res
    results = res.results
    kp = np.zeros((2048, 2), np.float32)
    desc = np.zeros((2048, 256), np.float32)
    for c in range(NCORES):
        kp += results[c]["out_kp"]
        desc += results[c]["out_desc"]
    scores = results[0]["out_scores"]
    det = results[0]["out_det"]
    return kp, desc, scores, det[0], det[1], det[2]


# revision 32
# speedup vs baseline: 1.0084x; 1.0084x over previous
"""Trainium2 Bass kernel for nn_ExtractionModel (NMS keypoint detection).

Self-contained: builds one SPMD Bass program, runs it on 8 NeuronCores via
run_bass_kernel_spmd, combines per-core partial outputs on the host.

Algorithm (per core):
  Phase A (replicated): NMS + Newton localization on the 3 score maps.
  Phase B (replicated): threshold-pruned candidate extraction (max8/match_replace)
    -> int32 keys (value<<12 | slot tie-break) -> bitonic top-2048 sort.
  Phase C (band-sharded): each core gathers/normalizes descriptors only for the
    winners inside its 72-row band of the 576-row virtual stack, scattering rows
    into the output by rank. Host sums the per-core partial kp/desc outputs.
"""
import numpy as np

H = W = 192
T0 = 0.974
M0 = 8170505  # ceil(T0 * 2^23)
FRAME = 74 * 192  # 14208
NCORES = 8

_CACHE = {}


def _build_program():
    from contextlib import ExitStack
    from concourse import bass, bacc, mybir, tile

    f32 = mybir.dt.float32
    i32 = mybir.dt.int32
    i16 = mybir.dt.int16
    u16 = mybir.dt.uint16
    u32 = mybir.dt.uint32
    A = mybir.AluOpType

    nc = bacc.Bacc(None, target_bir_lowering=False, debug=True)

    # ---------------- I/O ----------------
    scores_in = [nc.declare_dram_parameter(f"score{s}", [H, W], f32, isOutput=False)
                 for s in range(3)]
    feats_in = {(f, t): nc.declare_dram_parameter(f"feat{f}{t}", [128, FRAME], f32,
                                                  isOutput=False)
                for f in range(2) for t in range(2)}
    # per-core constants [128, 8]: c0=cboff0 c1=cboff1 c2=b0 c3=b1
    consts_in = nc.declare_dram_parameter("consts", [128, 8], f32, isOutput=False)

    out_kp = nc.declare_dram_parameter("out_kp", [2048, 2], f32, isOutput=True)
    out_desc = nc.declare_dram_parameter("out_desc", [2048, 256], f32, isOutput=True)
    out_scores = nc.declare_dram_parameter("out_scores", [2048], f32, isOutput=True)
    out_det = nc.declare_dram_parameter("out_det", [3, H, W], f32, isOutput=True)
    out_dbg = nc.declare_dram_parameter("out_dbg", [128, 32], mybir.dt.int32, isOutput=True)

    # ---------------- inline constants ----------------
    prow = nc.inline_tensor(np.arange(128, dtype=np.float32).reshape(128, 1), "prow")
    coliota = nc.inline_tensor(
        np.broadcast_to(np.arange(W, dtype=np.float32), (128, W)).copy(), "coliota")
    ident = nc.inline_tensor(np.eye(128, dtype=np.float32), "ident")
    ones128 = nc.inline_tensor(np.ones((128, 1), np.float32), "ones128")
    onesrow = nc.inline_tensor(np.ones((1, 128), np.float32), "onesrow")
    slotiota_np = (np.arange(128)[:, None] * 32 + np.arange(32)[None, :])
    slotiota = nc.inline_tensor(slotiota_np.astype(np.float32), "slotiota")
    e_np = np.arange(128)[:, None] * 32 + np.arange(32)[None, :]
    sgn_list = []
    for ki_ in range(1, 12):  # stages k=2..2048 (k=4096 is all +1)
        k = 2 ** ki_
        sgn_list.append(np.where((e_np & k) != 0, -1, 1).astype(np.float32))
    sgnmasks = nc.inline_tensor(np.concatenate(sgn_list, axis=1), "sgnmasks")  # [128, 352]
    rankiota = nc.inline_tensor(
        (np.arange(128)[:, None] * 32 + np.arange(32)[None, :]).astype(np.float32),
        "rankiota")
    rankrow = nc.inline_tensor(np.arange(2048, dtype=np.float32).reshape(1, 2048),
                               "rankrow")
    halfiota = nc.inline_tensor(np.arange(16, dtype=np.float32).reshape(16, 1),
                                "halfiota")
    onesr16 = nc.inline_tensor(np.ones((1, 16), np.float32), "onesr16")
    wiota16 = nc.inline_tensor(
        (np.arange(32)[None, :] * 16 + np.arange(16)[:, None]).astype(np.float32),
        "wiota16")

    with tile.TileContext(nc) as tc, ExitStack() as ctx:
        pconst = ctx.enter_context(tc.tile_pool(name="pconst", bufs=1))
        psum = ctx.enter_context(tc.tile_pool(name="psum", bufs=1, space="PSUM"))
        dram = ctx.enter_context(tc.tile_pool(name="dram", bufs=1, space="DRAM"))
        es1 = ExitStack()
        sb = es1.enter_context(tc.tile_pool(name="sbA", bufs=2))
        sb1 = es1.enter_context(tc.tile_pool(name="sb1A", bufs=1))
        sbd = es1.enter_context(tc.tile_pool(name="sbdA", bufs=2))

        V = nc.vector
        G = nc.gpsimd
        S = nc.scalar

        canddram = dram.tile([576, W], f32)
        kjud = dram.tile([576, W], f32)
        kiud = dram.tile([576, W], f32)
        cbd = dram.tile([576, W], f32)
        wid = dram.tile([576, W], f32)
        wjd = dram.tile([576, W], f32)
        locdram = dram.tile([128, 32], f32)
        mdram = dram.tile([4, 2048], f32)
        wdram = dram.tile([3, 512], f32)
        rkdram = dram.tile([512], f32)
        slotdram = dram.tile([2048], i32)
        seldram = dram.tile([2048], f32)
        lrowdram = dram.tile([2048], f32)
        lidxdram = dram.tile([2048], f32)
        cbcdram = dram.tile([16, 32], f32)
        descscr = dram.tile([2048, 256], f32)

        # load constants to SBUF
        constt = pconst.tile([128, 8], f32)
        nc.sync.dma_start(out=constt[:], in_=consts_in[:])
        prowt = pconst.tile([128, 1], f32)
        nc.sync.dma_start(out=prowt[:], in_=prow[:])
        colt = pconst.tile([128, W], f32)
        nc.sync.dma_start(out=colt[:], in_=coliota[:])
        identt = pconst.tile([128, 128], f32)
        nc.sync.dma_start(out=identt[:], in_=ident[:])
        ones_t = pconst.tile([128, 1], f32)
        nc.sync.dma_start(out=ones_t[:], in_=ones128[:])
        onesr_t = pconst.tile([1, 128], f32)
        nc.sync.dma_start(out=onesr_t[:], in_=onesrow[:])
        slotnt = sb1.tile([128, 32], f32)
        nc.sync.dma_start(out=slotnt[:], in_=slotiota[:])
        sgnt = sb1.tile([128, 352], f32)
        nc.sync.dma_start(out=sgnt[:], in_=sgnmasks[:])
        halfit = pconst.tile([16, 1], f32)
        nc.sync.dma_start(out=halfit[:], in_=halfiota[:])
        onesr16t = pconst.tile([1, 16], f32)
        nc.sync.dma_start(out=onesr16t[:], in_=onesr16[:])


        # ================= PHASE A =================
        for s in range(3):
            for v in range(2):
                r0 = 96 * v  # first interior image row of this vtile
                # three row-shifted, zero-padded copies (base partition 0)
                tls = {}
                for nm, shift in (("scC", 0), ("scU", 1), ("scD", -1)):
                    tl = sb.tile([96, W + 2], f32, tag=nm, name=nm)
                    V.memset(tl[:], 0.0)
                    lo = max(0, r0 + shift)
                    hi = min(192, r0 + 96 + shift)
                    plo = lo - (r0 + shift)
                    nc.sync.dma_start(out=tl[plo:plo + (hi - lo), 1:W + 1],
                                      in_=scores_in[s][lo:hi, :])
                    tls[nm] = tl
                scC, scU, scD = tls["scC"], tls["scU"], tls["scD"]
                inter = scC[:, 1:W + 1]

                h3c = sb.tile([96, W], f32, tag="h3c")
                V.tensor_max(h3c[:], scC[:, 0:W], scC[:, 1:W + 1])
                V.tensor_max(h3c[:], h3c[:], scC[:, 2:W + 2])
                h3u = sb.tile([96, W], f32, tag="h3u")
                V.tensor_max(h3u[:], scU[:, 0:W], scU[:, 1:W + 1])
                V.tensor_max(h3u[:], h3u[:], scU[:, 2:W + 2])
                lm = sb.tile([96, W], f32, tag="lm")
                V.tensor_max(lm[:], scD[:, 0:W], scD[:, 1:W + 1])
                V.tensor_max(lm[:], lm[:], scD[:, 2:W + 2])
                V.tensor_max(lm[:], lm[:], h3u[:])
                V.tensor_max(lm[:], lm[:], h3c[:])
                eq = sb.tile([96, W], f32, tag="eq")
                V.tensor_tensor(eq[:], lm[:], inter, op=A.is_equal)
                det = sb.tile([96, W], f32, tag="det")
                V.tensor_mul(det[:], inter, eq[:])
                nc.sync.dma_start(out=out_det[s, r0:r0 + 96, :], in_=det[:])

                di = sb.tile([96, W], f32, tag="di")
                V.tensor_sub(di[:], scU[:, 1:W + 1], scD[:, 1:W + 1])
                V.tensor_scalar_mul(di[:], di[:], 0.5)
                dj = sb.tile([96, W], f32, tag="dj")
                V.tensor_sub(dj[:], scC[:, 2:W + 2], scC[:, 0:W])
                V.tensor_scalar_mul(dj[:], dj[:], 0.5)
                dii = sb.tile([96, W], f32, tag="dii")
                V.tensor_add(dii[:], scD[:, 1:W + 1], scU[:, 1:W + 1])
                V.scalar_tensor_tensor(out=dii[:], in0=inter, scalar=-2.0,
                                       in1=dii[:], op0=A.mult, op1=A.add)
                djj = sb.tile([96, W], f32, tag="djj")
                V.tensor_add(djj[:], scC[:, 0:W], scC[:, 2:W + 2])
                V.scalar_tensor_tensor(out=djj[:], in0=inter, scalar=-2.0,
                                       in1=djj[:], op0=A.mult, op1=A.add)
                dij = sb.tile([96, W], f32, tag="dij")
                tb = sb.tile([96, W], f32, tag="tb")
                V.tensor_sub(dij[:], scD[:, 0:W], scD[:, 2:W + 2])
                V.tensor_sub(tb[:], scU[:, 2:W + 2], scU[:, 0:W])
                V.tensor_add(dij[:], dij[:], tb[:])
                V.tensor_scalar_mul(dij[:], dij[:], 0.25)

                dd = sb.tile([96, W], f32, tag="dd")
                V.tensor_mul(dd[:], dii[:], djj[:])
                V.tensor_mul(tb[:], dij[:], dij[:])
                V.tensor_sub(dd[:], dd[:], tb[:])
                ddz = sb.tile([96, W], f32, tag="ddz")
                V.tensor_scalar(ddz[:], dd[:], 0.0, None, op0=A.is_equal)
                V.scalar_tensor_tensor(out=dd[:], in0=ddz[:], scalar=1e-20,
                                       in1=dd[:], op0=A.mult, op1=A.add)
                rcp = sb.tile([96, W], f32, tag="rcp")
                V.reciprocal(rcp[:], dd[:])

                si = sb.tile([96, W], f32, tag="si")
                V.tensor_mul(si[:], djj[:], di[:])
                V.tensor_mul(tb[:], dij[:], dj[:])
                V.tensor_sub(si[:], tb[:], si[:])
                V.tensor_mul(si[:], si[:], rcp[:])
                sj = sb.tile([96, W], f32, tag="sj")
                V.tensor_mul(sj[:], dii[:], dj[:])
                V.tensor_mul(tb[:], dij[:], di[:])
                V.tensor_sub(sj[:], tb[:], sj[:])
                V.tensor_mul(sj[:], sj[:], rcp[:])

                valid = sb.tile([96, W], f32, tag="valid")
                V.tensor_scalar(valid[:], det[:], 0.0, None, op0=A.not_equal)
                S.activation(out=tb[:], in_=si[:],
                             func=mybir.ActivationFunctionType.Abs)
                V.tensor_scalar(tb[:], tb[:], 0.5, None, op0=A.is_lt)
                V.tensor_mul(valid[:], valid[:], tb[:])
                S.activation(out=tb[:], in_=sj[:],
                             func=mybir.ActivationFunctionType.Abs)
                V.tensor_scalar(tb[:], tb[:], 0.5, None, op0=A.is_lt)
                V.tensor_mul(valid[:], valid[:], tb[:])

                iiS = sb.tile([96, 1], f32, tag="iiS")
                V.tensor_scalar(iiS[:], prowt[0:96, :], float(r0), None, op0=A.add)
                ki = sb.tile([96, W], f32, tag="ki")
                V.tensor_scalar(ki[:], si[:], iiS[:], None, op0=A.add)
                V.tensor_mul(ki[:], ki[:], valid[:])
                kj = sb.tile([96, W], f32, tag="kj")
                V.tensor_add(kj[:], sj[:], colt[0:96, :])
                V.tensor_mul(kj[:], kj[:], valid[:])

                i0 = sb.tile([96, W], f32, tag="i0")
                V.tensor_scalar(i0[:], ki[:], iiS[:], None, op0=A.is_lt)
                V.tensor_scalar(i0[:], i0[:], -1.0, iiS[:], op0=A.mult, op1=A.add)
                j0 = sb.tile([96, W], f32, tag="j0")
                V.tensor_tensor(j0[:], kj[:], colt[0:96, :], op=A.is_lt)
                V.scalar_tensor_tensor(out=j0[:], in0=j0[:], scalar=-1.0,
                                       in1=colt[0:96, :], op0=A.mult, op1=A.add)
                V.tensor_scalar(tb[:], i0[:], 0.0, None, op0=A.is_ge)
                V.tensor_mul(valid[:], valid[:], tb[:])
                V.tensor_scalar(tb[:], i0[:], float(H - 2), None, op0=A.is_le)
                V.tensor_mul(valid[:], valid[:], tb[:])
                V.tensor_scalar(tb[:], j0[:], 0.0, None, op0=A.is_ge)
                V.tensor_mul(valid[:], valid[:], tb[:])
                V.tensor_scalar(tb[:], j0[:], float(W - 2), None, op0=A.is_le)
                V.tensor_mul(valid[:], valid[:], tb[:])

                V.tensor_scalar(i0[:], i0[:], 0.0, float(H - 2), op0=A.max, op1=A.min)
                V.tensor_scalar(j0[:], j0[:], 0.0, float(W - 2), op0=A.max, op1=A.min)
                wi = sb.tile([96, W], f32, tag="wi")
                V.tensor_sub(wi[:], ki[:], i0[:])
                wj = sb.tile([96, W], f32, tag="wj")
                V.tensor_sub(wj[:], kj[:], j0[:])
                cb = sb.tile([96, W], f32, tag="cb")
                V.tensor_scalar(cb[:], i0[:], 192.0, float(s * 36864),
                                op0=A.mult, op1=A.add)
                V.tensor_add(cb[:], cb[:], j0[:])
                kju = sb.tile([96, W], f32, tag="kju")
                V.tensor_scalar(kju[:], kj[:], 16.0, 7.5, op0=A.mult, op1=A.add)
                kiu = sb.tile([96, W], f32, tag="kiu")
                V.tensor_scalar(kiu[:], ki[:], 16.0, 7.5, op0=A.mult, op1=A.add)

                c1 = sb.tile([96, W], f32, tag="c1")
                V.tensor_scalar(c1[:], det[:], T0, None, op0=A.is_gt)
                V.tensor_mul(c1[:], c1[:], valid[:])
                cand = sb.tile([96, W], f32, tag="cand")
                V.tensor_mul(cand[:], det[:], c1[:])
                V.tensor_add(cand[:], cand[:], c1[:])
                V.tensor_scalar(cand[:], cand[:], 1.0, None, op0=A.subtract)

                row0 = s * 192 + r0
                nc.sync.dma_start(out=canddram[row0:row0 + 96, :], in_=cand[:])
                nc.sync.dma_start(out=kjud[row0:row0 + 96, :], in_=kju[:])
                nc.sync.dma_start(out=kiud[row0:row0 + 96, :], in_=kiu[:])
                nc.sync.dma_start(out=cbd[row0:row0 + 96, :], in_=cb[:])
                nc.sync.dma_start(out=wid[row0:row0 + 96, :], in_=wi[:])
                nc.sync.dma_start(out=wjd[row0:row0 + 96, :], in_=wj[:])

        # ================= PHASE B: extraction =================
        candt = sb1.tile([128, 864], f32)
        nc.sync.dma_start(out=candt[:], in_=canddram[:].rearrange("a b -> (a b)")
                          .rearrange("(p f) -> p f", p=128))
        vals = sb1.tile([128, 32], f32)
        locs = sb1.tile([128, 32], f32)
        p864 = sb1.tile([128, 1], f32)
        V.tensor_scalar(p864[:], prowt[:], 864.0, None, op0=A.mult)
        for h in range(2):
            work = sb.tile([128, 432], f32, tag="work")
            V.tensor_copy(work[:], candt[:, 432 * h:432 * h + 432])
            for r in range(2):
                mx = sb.tile([128, 8], f32, tag="mx")
                V.max(out=mx[:], in_=work[:])
                mi = sb.tile([128, 8], u16, tag="mi")
                V.max_index(mi[:], mx[:], work[:])
                V.tensor_copy(vals[:, 16 * h + 8 * r:16 * h + 8 * r + 8], mx[:])
                mif = sb.tile([128, 8], f32, tag="mif")
                V.tensor_copy(mif[:], mi[:])
                V.tensor_scalar(locs[:, 16 * h + 8 * r:16 * h + 8 * r + 8],
                                mif[:], p864[:], float(432 * h),
                                op0=A.add, op1=A.add)
                if r == 0:
                    work2 = sb.tile([128, 432], f32, tag="work2")
                    V.match_replace(out=work2[:], in_to_replace=mx[:],
                                    in_values=work[:], imm_value=-1.0)
                    work = work2
        nc.sync.dma_start(out=locdram[:], in_=locs[:])

        # keys
        key = sbd.tile([128, 32], f32, tag="key128")
        V.tensor_scalar(key[:], vals[:], float(2.0 ** 23), float(-M0),
                        op0=A.mult, op1=A.add)
        V.tensor_scalar_max(key[:], key[:], -1.0)
        pay = sbd.tile([128, 32], f32, tag="pay128")
        V.tensor_copy(pay[:], slotnt[:])

        # ================= bitonic sort (descending, key+payload) =================
        def pass_free(sk, sp, j, P, F):
            # compare-exchange at free distance j on [P, F] tiles
            dk = sbd.tile([P, F], f32, tag=f"key{P}", name="dk")
            dp = sbd.tile([P, F], f32, tag=f"pay{P}", name="dp")
            sv = sk[:].rearrange("p (g two r) -> p g two r", two=2, r=j)
            pv = sp[:].rearrange("p (g two r) -> p g two r", two=2, r=j)
            dv = dk[:].rearrange("p (g two r) -> p g two r", two=2, r=j)
            qv = dp[:].rearrange("p (g two r) -> p g two r", two=2, r=j)
            pr = sb.tile([P, F], i32, tag=f"pr{P}", name="pr")
            prv = pr[:].rearrange("p (g two r) -> p g two r", two=2, r=j)[:, :, 0, :]
            V.tensor_tensor(prv[:], sv[:, :, 0, :], sv[:, :, 1, :], op=A.is_ge)
            V.tensor_tensor(dv[:, :, 0, :], sv[:, :, 0, :], sv[:, :, 1, :], op=A.max)
            V.tensor_tensor(dv[:, :, 1, :], sv[:, :, 0, :], sv[:, :, 1, :], op=A.min)
            V.tensor_copy(qv[:, :, 0, :], pv[:, :, 1, :])
            V.copy_predicated(qv[:, :, 0, :], prv[:], pv[:, :, 0, :])
            V.tensor_copy(qv[:, :, 1, :], pv[:, :, 0, :])
            V.copy_predicated(qv[:, :, 1, :], prv[:], pv[:, :, 1, :])
            return dk, dp

        def pe_transpose(src_ap, P, F, tagn):
            # [P, F] -> [F, P] via PE transpose
            pst = psum.tile([F, P], f32, tag=f"pst{F}", name="pst")
            nc.tensor.transpose(out=pst[:], in_=src_ap, identity=identt[0:P, 0:P])
            ot = sbd.tile([F, P], f32, tag=tagn, name="ot")
            V.tensor_copy(ot[:], pst[:])
            return ot

        cur, curp = key, pay
        for ki_ in range(1, 13):
            k = 2 ** ki_
            if k < 4096:
                sg = sgnt[:, 32 * (ki_ - 1):32 * ki_]
                nxt = sbd.tile([128, 32], f32, tag="key128", name="nxt")
                V.tensor_mul(nxt[:], cur[:], sg)
                cur = nxt
            js = []
            j = k // 2
            while j >= 1:
                js.append(j)
                j //= 2
            jbig = [j for j in js if j >= 1024]
            jmid = [j for j in js if 32 <= j <= 512]
            jsml = [j for j in js if j <= 16]
            if jbig:
                Tk = pe_transpose(cur[:], 128, 32, "keyT")
                Tp = pe_transpose(curp[:], 128, 32, "payT")
                for j in jbig:
                    Tk, Tp = pass_free(Tk, Tp, j // 32, 32, 128)
                cur = pe_transpose(Tk[:], 32, 128, "key128")
                curp = pe_transpose(Tp[:], 32, 128, "pay128")
            if jmid:
                Bk = sbd.tile([128, 32], f32, tag="key128", name="Bk")
                V.transpose(Bk[:], cur[:])
                Bp = sbd.tile([128, 32], f32, tag="pay128", name="Bp")
                V.transpose(Bp[:], curp[:])
                cur, curp = Bk, Bp
                for j in jmid:
                    cur, curp = pass_free(cur, curp, j // 32, 128, 32)
                Bk2 = sbd.tile([128, 32], f32, tag="key128", name="Bk2")
                V.transpose(Bk2[:], cur[:])
                Bp2 = sbd.tile([128, 32], f32, tag="pay128", name="Bp2")
                V.transpose(Bp2[:], curp[:])
                cur, curp = Bk2, Bp2
            for j in jsml:
                cur, curp = pass_free(cur, curp, j, 128, 32)
            if k < 4096:
                sg = sgnt[:, 32 * (ki_ - 1):32 * ki_]
                nxt = sbd.tile([128, 32], f32, tag="key128", name="nxt2")
                V.tensor_mul(nxt[:], cur[:], sg)
                cur = nxt

        # ============ tie fix: ascending slots within equal-key runs ============
        kL = sb1.tile([128, 32], f32)
        V.memset(kL[:, 0:1], 3.0e8)
        V.tensor_copy(kL[:, 1:32], cur[:, 0:31])
        nc.sync.dma_start(out=kL[1:128, 0:1], in_=cur[0:127, 31:32])
        pL = sb1.tile([128, 32], f32)
        V.memset(pL[:, 0:1], 0.0)
        V.tensor_copy(pL[:, 1:32], curp[:, 0:31])
        nc.sync.dma_start(out=pL[1:128, 0:1], in_=curp[0:127, 31:32])
        kR = sb1.tile([128, 32], f32)
        V.memset(kR[:, 31:32], -2.0)
        V.tensor_copy(kR[:, 0:31], cur[:, 1:32])
        nc.sync.dma_start(out=kR[0:127, 31:32], in_=cur[1:128, 0:1])
        pR = sb1.tile([128, 32], f32)
        V.memset(pR[:, 31:32], 0.0)
        V.tensor_copy(pR[:, 0:31], curp[:, 1:32])
        nc.sync.dma_start(out=pR[0:127, 31:32], in_=curp[1:128, 0:1])
        eqL = sb1.tile([128, 32], i32)
        V.tensor_tensor(eqL[:], cur[:], kL[:], op=A.is_equal)
        eqR = sb1.tile([128, 32], i32)
        V.tensor_tensor(eqR[:], cur[:], kR[:], op=A.is_equal)
        tmpn = sb1.tile([128, 32], f32)
        V.tensor_tensor(tmpn[:], curp[:], pR[:], op=A.min)
        tmpx = sb1.tile([128, 32], f32)
        V.tensor_tensor(tmpx[:], curp[:], pL[:], op=A.max)
        newp = sb1.tile([128, 32], f32)
        V.tensor_copy(newp[:], curp[:])
        V.copy_predicated(newp[:], eqR[:], tmpn[:])
        V.copy_predicated(newp[:], eqL[:], tmpx[:])
        curp = newp

        # ================= decode winners =================
        wk = cur[0:64, :]
        vf = sb1.tile([64, 32], f32)
        V.tensor_scalar(vf[:], wk, float(M0), None, op0=A.add)
        V.tensor_scalar(vf[:], vf[:], float(2.0 ** -23), None, op0=A.mult)
        sel = sb1.tile([64, 32], f32)
        V.tensor_scalar(sel[:], wk, 0.0, None, op0=A.is_ge)
        scw = sb1.tile([64, 32], f32)
        V.tensor_mul(scw[:], vf[:], sel[:])
        nc.sync.dma_start(out=out_scores[:], in_=scw[:])

        slotf = sb1.tile([64, 32], f32)
        V.tensor_mul(slotf[:], curp[0:64, :], sel[:])
        slott = sb1.tile([64, 32], i32)
        V.tensor_copy(slott[:], slotf[:])
        nc.sync.dma_start(out=out_dbg[0:64, :], in_=slott[:])

        nc.sync.dma_start(out=slotdram[:], in_=slott[:])
        nc.sync.dma_start(out=seldram[:], in_=sel[:])

        # ======== PHASE 2: winner attribute gathers (row space) ========
        es1.close()
        es2 = ExitStack()
        sb = es2.enter_context(tc.tile_pool(name="sbB", bufs=1))
        sb1 = es2.enter_context(tc.tile_pool(name="sb1B", bufs=1))
        rankrt = sb1.tile([1, 2048], f32)
        nc.sync.dma_start(out=rankrt[:], in_=rankrow[:])
        sw32 = sb1.tile([16, 128], i32)
        nc.sync.dma_start(out=sw32[:],
                          in_=slotdram[:].rearrange("(f p) -> p f", p=16))
        sw16 = sb1.tile([16, 128], i16)
        V.tensor_copy(sw16[:], sw32[:])

        # replicate loc list to 16 partitions
        loc1 = sb1.tile([16, 4096], f32)
        nc.sync.dma_start(out=loc1[0:1, :],
                          in_=locdram[:].rearrange("a b -> (a b)"))
        for db in (1, 2, 4, 8):
            nc.sync.dma_start(out=loc1[db:2 * db, :], in_=loc1[0:db, :])

        lw16 = sb1.tile([16, 2048], f32)
        G.ap_gather(out_ap=lw16[:].unsqueeze(2), in_ap=loc1[:].unsqueeze(2),
                    idxs_ap=sw16[:], channels=16, num_elems=4096, d=1,
                    num_idxs=2048)
        lrow = lw16[0:1, :]

        # half index h in [0,6) and local index within half
        hrow = sb1.tile([1, 2048], f32)
        V.tensor_scalar(hrow[:], lrow, 9216.0, None, op0=A.is_ge)
        for kk in range(2, 12):
            htmp = sb.tile([1, 2048], f32, tag="rtmp", name="htmp", bufs=2)
            V.tensor_scalar(htmp[:], lrow, float(9216 * kk), None, op0=A.is_ge)
            V.tensor_add(hrow[:], hrow[:], htmp[:])
        lidxrow = sb1.tile([1, 2048], f32)
        V.tensor_scalar(lidxrow[:], hrow[:], -9216.0, None, op0=A.mult)
        V.tensor_add(lidxrow[:], lidxrow[:], lrow)
        nc.sync.dma_start(out=lidxdram[:], in_=lidxrow[:])
        liw32 = sb1.tile([16, 128], f32)
        nc.sync.dma_start(out=liw32[:],
                          in_=lidxdram[:].rearrange("(f p) -> p f", p=16))
        liw16 = sb1.tile([16, 128], i16)
        V.tensor_copy(liw16[:], liw32[:])

        # sel to row space
        selrow = sb1.tile([1, 2048], f32)
        nc.sync.dma_start(out=selrow[:], in_=seldram[:])

        ownrow = sb.tile([1, 2048], f32, tag="rtmp", name="ownrow", bufs=2)
        owntmp = sb.tile([1, 2048], f32, tag="rtmp", name="owntmp", bufs=2)
        V.tensor_scalar(ownrow[:], lrow, constt[0:1, 2:3], None, op0=A.is_ge)
        V.tensor_scalar(owntmp[:], lrow, constt[0:1, 3:4], None, op0=A.is_lt)
        V.tensor_mul(ownrow[:], ownrow[:], owntmp[:])
        omrow = sb1.tile([1, 2048], f32)
        V.tensor_mul(omrow[:], ownrow[:], selrow[:])

        # broadcast h to 16 partitions via PE; build half mask
        hb = sb1.tile([16, 2048], f32)
        for ch in range(4):
            psb = psum.tile([16, 512], f32, tag="psb", name="psb")
            nc.tensor.matmul(out=psb[:], lhsT=onesr16t[:],
                             rhs=hrow[:, 512 * ch:512 * ch + 512],
                             start=True, stop=True)
            V.tensor_copy(hb[:, 512 * ch:512 * ch + 512], psb[:])
        hmask = sb1.tile([16, 2048], f32)
        V.tensor_scalar(hmask[:], hb[:], halfit[:], None, op0=A.is_equal)

        def attr_gather_row(src_dram, name):
            at = sb.tile([16, 9216], f32, tag="attrin", name="attrin")
            nc.sync.dma_start(
                out=at[0:12, :],
                in_=src_dram[:].rearrange("(h r) c -> h (r c)", h=12))
            nc.sync.dma_start(
                out=at[12:16, :],
                in_=src_dram[:].rearrange("(h r) c -> h (r c)", h=12)[0:4, :])
            g16 = sb.tile([16, 2048], f32, tag="attrg", name="attrg")
            G.ap_gather(out_ap=g16[:].unsqueeze(2), in_ap=at[:].unsqueeze(2),
                        idxs_ap=liw16[:], channels=16, num_elems=9216, d=1,
                        num_idxs=2048)
            V.tensor_mul(g16[:], g16[:], hmask[:])
            row = sb.tile([1, 2048], f32, tag="arow", name=f"arow_{name}", bufs=2)
            for ch in range(4):
                psr = psum.tile([1, 512], f32, tag="psr", name="psr")
                nc.tensor.matmul(out=psr[:], lhsT=ones_t[0:16, :],
                                 rhs=g16[:, 512 * ch:512 * ch + 512],
                                 start=True, stop=True)
                V.tensor_copy(row[:, 512 * ch:512 * ch + 512], psr[:])
            return row

        def mask_write(src_row, k):
            t = sb.tile([1, 2048], f32, tag="rtmp", name="mw", bufs=2)
            V.tensor_mul(t[:], src_row, omrow[:])
            V.tensor_add(t[:], t[:], omrow[:])
            V.tensor_scalar(t[:], t[:], 1.0, None, op0=A.subtract)
            nc.sync.dma_start(out=mdram[k, :], in_=t[:])

        row = attr_gather_row(kjud, "kju")
        kpr = sb.tile([1, 2048], f32, tag="rtmp", name="kpr", bufs=2)
        V.tensor_mul(kpr[:], row[:], omrow[:])
        nc.sync.dma_start(out=out_kp[:, 0:1], in_=kpr[:])
        row = attr_gather_row(kiud, "kiu")
        kpr = sb.tile([1, 2048], f32, tag="rtmp", name="kpr2", bufs=2)
        V.tensor_mul(kpr[:], row[:], omrow[:])
        nc.sync.dma_start(out=out_kp[:, 1:2], in_=kpr[:])
        row = attr_gather_row(cbd, "cb")
        mask_write(row[:], 0)
        mask_write(rankrt[:], 1)
        row = attr_gather_row(wid, "wi")
        mask_write(row[:], 2)
        row = attr_gather_row(wjd, "wj")
        mask_write(row[:], 3)

        # read back wrapped [16,128], sparse-compact to [16,32]
        wiotat = sb1.tile([16, 32], f32)
        nc.sync.dma_start(out=wiotat[:], in_=wiota16[:])
        comp = {}
        for k, name in ((0, "cb"), (1, "rk"), (2, "wi"), (3, "wj")):
            t = sb.tile([16, 128], f32, tag="cmpin")
            nc.sync.dma_start(out=t[:],
                              in_=mdram[k, :].rearrange("(f p) -> p f", p=16))
            o = sb1.tile([16, 32], f32, tag=f"comp_{name}")
            nf = sb.tile([1, 1], u32, tag="nf")
            G.sparse_gather(out=o[:], in_=t[:], num_found=nf[:])
            # deterministic -1 padding: positions >= num_found forced to -1
            nff = sb.tile([1, 1], f32, tag="nff")
            V.tensor_copy(nff[:], nf[:])
            nf16 = sb.tile([16, 1], f32, tag="nf16")
            nc.sync.dma_start(out=nf16[0:1, :], in_=nff[:])
            for db in (1, 2, 4, 8):
                nc.sync.dma_start(out=nf16[db:2 * db, :], in_=nf16[0:db, :])
            pdm = sb.tile([16, 32], i32, tag="pdm")
            V.tensor_scalar(pdm[:], wiotat[:], nf16[:], None, op0=A.is_lt)
            o2 = sb1.tile([16, 32], f32, tag=f"comp2_{name}", name=f"o2_{name}")
            V.memset(o2[:], -1.0)
            V.copy_predicated(o2[:], pdm[:], o[:])
            comp[name] = o2

        # rank offsets for the scatter: pad -> 60000 so bounds check skips
        rz = sb1.tile([16, 32], f32)
        V.tensor_scalar(rz[:], comp["rk"][:], 0.0, None, op0=A.is_lt)
        V.scalar_tensor_tensor(out=rz[:], in0=rz[:], scalar=60001.0,
                               in1=comp["rk"][:], op0=A.mult, op1=A.add)
        nc.sync.dma_start(out=rkdram[:].rearrange("(f p) -> p f", p=16), in_=rz[:])
        # weights roundtrip -> wdram, cbc handoff
        for k, name in ((0, "wi"), (1, "wj"), (2, "cb")):
            nc.sync.dma_start(out=wdram[k, :].rearrange("(f p) -> p f", p=16),
                              in_=comp[name][:])
        nc.sync.dma_start(out=cbcdram[:], in_=comp["cb"][:])

        # ======== PHASE 3: descriptors ========
        es2.close()
        es3 = ExitStack()
        sb = es3.enter_context(tc.tile_pool(name="sbC", bufs=1))
        sb1 = es3.enter_context(tc.tile_pool(name="sb1C", bufs=1))
        featp = es3.enter_context(tc.tile_pool(name="featp", bufs=1))
        cbct = sb1.tile([16, 32], f32)
        nc.sync.dma_start(out=cbct[:], in_=cbcdram[:])

        # corner index tiles per frag: [16,128] int16 wrapped, replicated to [128,128]
        idx16 = {}
        for f in range(2):
            base = sb.tile([16, 32], f32, tag="cbase")
            V.tensor_scalar(base[:], cbct[:], constt[0:16, f:f + 1], None,
                            op0=A.subtract)
            V.tensor_scalar(base[:], base[:], 0.0, float(FRAME - 194),
                            op0=A.max, op1=A.min)
            it = sb1.tile([128, 128], i16, tag=f"idx{f}")
            for ci, off in enumerate((0, 1, 192, 193)):
                cf = sb.tile([16, 32], f32, tag="cf")
                V.tensor_scalar(cf[:], base[:], float(off), None, op0=A.add)
                V.tensor_copy(it[0:16, 32 * ci:32 * ci + 32], cf[:])
            for rep in range(1, 8):
                nc.sync.dma_start(out=it[16 * rep:16 * rep + 16, :],
                                  in_=it[0:16, :])
            idx16[f] = it

        # weights rows from wdram
        wrow = [sb1.tile([1, 512], f32, tag=f"wrow{k}", name=f"wrow{k}")
                for k in range(3)]
        for k in range(3):
            nc.sync.dma_start(out=wrow[k][:], in_=wdram[k:k + 1, :])
        aw = sb1.tile([1, 512], f32)
        V.tensor_scalar(aw[:], wrow[0][:], -1.0, 1.0, op0=A.mult, op1=A.add)
        bw = sb1.tile([1, 512], f32)
        V.tensor_scalar(bw[:], wrow[1][:], -1.0, 1.0, op0=A.mult, op1=A.add)
        w4 = [sb1.tile([1, 512], f32, tag=f"w4_{ci}", name=f"w4_{ci}") for ci in range(4)]
        V.tensor_mul(w4[0][:], aw[:], bw[:])
        V.tensor_mul(w4[1][:], aw[:], wrow[1][:])
        V.tensor_mul(w4[2][:], wrow[0][:], bw[:])
        V.tensor_mul(w4[3][:], wrow[0][:], wrow[1][:])
        pmrow = sb1.tile([1, 512], f32)
        V.tensor_scalar(pmrow[:], wrow[2][:], 0.0, None, op0=A.is_ge)
        fm = [sb1.tile([1, 512], f32, tag=f"fm_{f}", name=f"fm_{f}") for f in range(2)]
        # fm_f = (0 <= cb - off) & (cb - off <= FRAME-194) & pm
        for f in range(2):
            t1 = sb.tile([1, 512], f32, tag="fmt")
            V.tensor_scalar(t1[:], wrow[2][:], constt[0:1, f:f + 1], None,
                            op0=A.subtract)
            t2 = sb.tile([1, 512], f32, tag="fmt2")
            V.tensor_scalar(t2[:], t1[:], 0.0, None, op0=A.is_ge)
            V.tensor_scalar(t1[:], t1[:], float(FRAME - 194), None, op0=A.is_le)
            V.tensor_mul(t1[:], t1[:], t2[:])
            V.tensor_mul(fm[f][:], t1[:], pmrow[:])

        # ================= descriptor gather + combine =================
        def bcast_row(row_ap, tagn):
            ps_b = psum.tile([128, 512], f32, tag="bps")
            nc.tensor.matmul(out=ps_b[:], lhsT=onesr_t[:], rhs=row_ap,
                             start=True, stop=True)
            ob = sb1.tile([128, 512], f32, tag=tagn)
            V.tensor_copy(ob[:], ps_b[:])
            return ob

        w4b = [bcast_row(w4[ci][:], f"w4b{ci}") for ci in range(4)]
        fmb = [bcast_row(fm[f][:], f"fmb{f}") for f in range(2)]
        desct = []
        for t in range(2):
            dt_ = sb1.tile([128, 512], f32, tag=f"desc{t}")
            V.memset(dt_[:], 0.0)
            desct.append(dt_)
        for f in range(2):
            ftiles = {}
            for t in range(2):
                ft = featp.tile([128, FRAME], f32, tag="featc")
                nc.sync.dma_start(out=ft[:], in_=feats_in[(f, t)][:])
                ftiles[t] = ft
            for t in range(2):
                g = sb.tile([128, 2048], f32, tag="gat")
                G.ap_gather(out_ap=g[:].unsqueeze(2), in_ap=ftiles[t][:].unsqueeze(2),
                            idxs_ap=idx16[f][:], channels=128, num_elems=FRAME,
                            d=1, num_idxs=2048)
                comb = sb.tile([128, 512], f32, tag="comb")
                V.tensor_mul(comb[:], g[:, 0:512], w4b[0][:])
                for ci in range(1, 4):
                    t3 = sb.tile([128, 512], f32, tag="combt")
                    V.tensor_mul(t3[:], g[:, 512 * ci:512 * ci + 512], w4b[ci][:])
                    V.tensor_add(comb[:], comb[:], t3[:])
                V.tensor_mul(comb[:], comb[:], fmb[f][:])
                V.tensor_add(desct[t][:], desct[t][:], comb[:])

        # ================= normalize =================
        nps = psum.tile([1, 512], f32, tag="nps")
        for t in range(2):
            sq = sb.tile([128, 512], f32, tag="sq")
            S.activation(out=sq[:], in_=desct[t][:],
                         func=mybir.ActivationFunctionType.Square)
            nc.tensor.matmul(out=nps[:], lhsT=ones_t[:], rhs=sq[:],
                             start=(t == 0), stop=(t == 1))
        nrm = sb1.tile([1, 512], f32)
        V.tensor_copy(nrm[:], nps[:])
        S.activation(out=nrm[:], in_=nrm[:], func=mybir.ActivationFunctionType.Sqrt)
        V.tensor_scalar_max(nrm[:], nrm[:], 1e-12)
        rcpn = sb1.tile([1, 512], f32)
        V.reciprocal(rcpn[:], nrm[:])
        rcb = bcast_row(rcpn[:], "rcb")
        for t in range(2):
            V.tensor_mul(desct[t][:], desct[t][:], rcb[:])

        # ================= transpose + rank scatter =================
        rkt = sb1.tile([128, 4], f32)
        nc.sync.dma_start(out=rkt[:], in_=rkdram[:].rearrange("(f p) -> p f", p=128))
        rki = sb1.tile([128, 4], i32)
        V.tensor_copy(rki[:], rkt[:])

        # zero-fill descriptor scratch
        dflat = descscr[:].rearrange("a b -> (a b)")
        for hh in range(2):
            zt = sb.tile([128, 2048], f32, tag="big")
            V.memset(zt[:], 0.0)
            nc.sync.dma_start(
                out=dflat[262144 * hh:262144 * (hh + 1)].rearrange(
                    "(p f) -> p f", p=128), in_=zt[:])

        for q in range(4):  # winner quarter (128 ranks each)
            ps = psum.tile([128, 128], f32, tag="pst")
            dT = sb.tile([128, 256], f32, tag="dT")
            for t in range(2):
                nc.tensor.transpose(out=ps[:], in_=desct[t][:, 128 * q:128 * q + 128],
                                    identity=identt[:])
                V.tensor_copy(dT[:, 128 * t:128 * t + 128], ps[:])
            G.indirect_dma_start(
                out=descscr[:], out_offset=bass.IndirectOffsetOnAxis(
                    ap=rki[:, q:q + 1], axis=0),
                in_=dT[:], in_offset=None,
                bounds_check=2047, oob_is_err=False)

        # copy scratch -> output
        oflat = out_desc[:].rearrange("a b -> (a b)")
        for hh in range(2):
            fin = sb.tile([128, 2048], f32, tag="big")
            nc.sync.dma_start(
                out=fin[:], in_=dflat[262144 * hh:262144 * (hh + 1)].rearrange(
                    "(p f) -> p f", p=128))
            nc.sync.dma_start(
                out=oflat[262144 * hh:262144 * (hh + 1)].rearrange(
                    "(p f) -> p f", p=128), in_=fin[:])
        es3.close()

    nc.compile()
    return nc


def _make_inputs_per_core(inputs):
    feats = [np.ascontiguousarray(inputs[k][0]) for k in
             ("feat_early", "feat_middle", "feat_deep")]
    smaps = [np.ascontiguousarray(inputs[k][0, 0], dtype=np.float32) for k in
             ("score_early", "score_middle", "score_deep")]
    in_maps = []
    for c in range(NCORES):
        m = {f"score{s}": smaps[s] for s in range(3)}
        r0 = 72 * c
        s0, off0 = r0 // 192, r0 % 192
        n1 = min(192 - off0, 72)
        frags = [(s0, off0)]
        frags.append((s0 + 1, 0) if n1 < 72 else (None, None))
        cboffs = []
        for (scl, off) in frags:
            if scl is None or scl > 2:
                cboffs.append(np.float32(1e9))
            else:
                cboffs.append(np.float32(scl * 36864 + (off - 1) * 192))
        for f, (scl, off) in enumerate(frags):
            if scl is None or scl > 2:
                fr = np.zeros((256, 74, 192), np.float32)
            else:
                fr = np.zeros((256, 74, 192), np.float32)
                lo, hi_r = off - 1, off + 73
                slo, shi = max(lo, 0), min(hi_r, 192)
                fr[:, slo - lo: slo - lo + (shi - slo)] = feats[scl][:, slo:shi]
            fr = fr.reshape(256, FRAME)
            for t in range(2):
                m[f"feat{f}{t}"] = np.ascontiguousarray(fr[128 * t:128 * t + 128])
        consts = np.zeros((128, 8), np.float32)
        consts[:, 0] = cboffs[0]
        consts[:, 1] = cboffs[1]
        consts[:, 2] = np.float32(13824 * c)
        consts[:, 3] = np.float32(13824 * (c + 1))
        m["consts"] = consts
        in_maps.append(m)
    return in_maps


def kernel(**inputs):
    if "nc" not in _CACHE:
        _CACHE["nc"] = _build_program()
    nc = _CACHE["nc"]
    in_maps = _make_inputs_per_core(inputs)
    from concourse.bass_utils import run_bass_kernel_spmd
    import os
    res = run_bass_kernel_spmd(nc, in_maps, list(range(NCORES)),
                               trace=bool(os.environ.get("KERNEL_TRACE")))
    _CACHE["last_result"] = res
    results = res.results
    kp = np.zeros((2048, 2), np.float32)
    desc = np.zeros((2048, 256), np.float32)
    for c in range(NCORES):
        kp += results[c]["out_kp"]
        desc += results[c]["out_desc"]
    scores = results[0]["out_scores"]
    det = results[0]["out_det"]
    return kp, desc, scores, det[0], det[1], det[2]
